# revision 5
# baseline (speedup 1.0000x reference)
"""Trainium2 Bass kernel for nn_Attention_40510131535961.

The reference module applies softmax over a size-1 axis, so the attention
weights are identically 1.0 and the whole attn MLP (W1/b1/W2/b2, LeakyReLU)
is dead code.  The output reduces to

    context[b, 0, e] = sum_s encode_output[b, s, e]        # [32, 1, 1024]

Strategy: data-parallel over batch across 8 NeuronCores (4 batches/core).
Per core, stream the [4, 2048, 1024] f32 shard through SBUF in 2 MiB DMAs
([128 s-partitions, 4 s-subchunks, 1024 e] tiles), accumulate on VectorE
(fp32 tensor_tensor adds), fold to [128, 1024] per batch, then reduce the
partition axis with a ones-vector matmul on TensorE into PSUM.  The kernel
is HBM-bound: ~32 MiB/core @ ~358 GB/s ≈ 90 us.
"""

import numpy as np

import concourse.bacc as bacc
import concourse.bass as bass
import concourse.mybir as mybir
import concourse.tile as tile
from concourse.bass_utils import run_bass_kernel_spmd

N_CORES = 8
B, S, E = 32, 2048, 1024
BP = B // N_CORES      # batches per core
P = 128                # SBUF partitions
CHUNKS = 4             # DMA chunks per batch
KSUB = S // (CHUNKS * P)  # s-subchunks per chunk (free-dim groups)
F32 = mybir.dt.float32

_CACHE = {}


def _build_nc() -> bass.Bass:
    # Bacc (not raw Bass): its compile()/finalize() runs
    # generate_event_semaphores(), which splits multi-sem waits into
    # InstEventSemaphore — TRN2 instructions support at most 1 wait.
    nc = bacc.Bacc()
    x = nc.declare_dram_parameter("x", [BP, S, E], F32, isOutput=False)
    y = nc.declare_dram_parameter("y", [BP, E], F32, isOutput=True)

    # s = c*(KSUB*P) + k*P + p  ->  per-(b,c) DMA of [p=128, k=KSUB, e=E]
    xr = x[:].rearrange("b (c k p) e -> b c p k e", c=CHUNKS, k=KSUB, p=P)

    with tile.TileContext(nc) as tc:
        with (
            tc.tile_pool(name="inp", bufs=8) as pin,
            tc.tile_pool(name="small", bufs=1) as psm,
            tc.tile_pool(name="ps", bufs=4, space="PSUM") as pps,
        ):
            ones = psm.tile([P, 1], F32)
            nc.vector.memset(ones[:], 1.0)
            out_sb = psm.tile([1, BP * E], F32)

            for b in range(BP):
                acc = None
                for c in range(CHUNKS):
                    t = pin.tile([P, KSUB, E], F32, tag="inp")
                    nc.sync.dma_start(t[:], xr[b, c])
                    flat = t[:].rearrange("p k e -> p (k e)")
                    if acc is None:
                        acc = flat
                    else:
                        nc.vector.tensor_add(acc, acc, flat)
                # fold KSUB*E -> E within partitions
                w = KSUB * E
                while w > E:
                    w //= 2
                    nc.vector.tensor_add(acc[:, :w], acc[:, :w], acc[:, w : 2 * w])
                # partition-axis reduce via ones-matmul (fp32, N<=512/bank)
                for h in range(E // 512):
                    ps = pps.tile([1, 512], F32, tag="ps")
                    nc.tensor.matmul(ps[:], ones[:], acc[:, h * 512 : (h + 1) * 512])
                    nc.any.tensor_copy(
                        out_sb[:, b * E + h * 512 : b * E + (h + 1) * 512], ps[:]
                    )

            # keep both APs 2D: 1D DRAM APs break NEFF load on this stack
            nc.sync.dma_start(y[:].rearrange("b e -> (b e)")[None, :], out_sb[:1, :])
    return nc


def _get_nc() -> bass.Bass:
    if "nc" not in _CACHE:
        nc = _build_nc()
        nc.finalize()
        _CACHE["nc"] = nc
    return _CACHE["nc"]


def _run(encode_output: np.ndarray, **spmd_kwargs):
    enc = np.ascontiguousarray(np.asarray(encode_output, dtype=np.float32))
    assert enc.shape == (B, S, E), enc.shape
    in_maps = [{"x": enc[i * BP : (i + 1) * BP]} for i in range(N_CORES)]
    res = run_bass_kernel_spmd(_get_nc(), in_maps, list(range(N_CORES)), **spmd_kwargs)
    out = np.concatenate([res.results[i]["y"] for i in range(N_CORES)], axis=0)
    return out.reshape(B, 1, E), res


def kernel(encode_output, hidden_state=None, W1=None, b1=None, W2=None, b2=None):
    out, _ = _run(encode_output)
    return out
